# revision 36
# baseline (speedup 1.0000x reference)
"""Bass/Trainium2 kernel for nn_BigramLanguageModel (v3).

Sharding (8 NeuronCores, single SPMD launch, no collectives):
  - core j: batch b = j//4, vocab quarter q = j%4. Each core runs the
    3-layer transformer on its 1024-token batch (2-way data parallel)
    and computes logits[:, 12800*q : 12800*(q+1)] (4-way tensor
    parallel over the padded 51200 vocab). Host concatenates.
  - All matmul operands are bf16 (fp32 PSUM accumulation, fp32 residual
    stream h in SBUF). Logits leave the core as bf16 (halves the output
    DMA) and are upcast on host. rel-err budget 2e-2 >> bf16 ~2e-3.
  - LayerNorm affine is folded into the following projections
    host-side; 1/sqrt(HS) into Wk; q/k biases into the PSUM
    evacuations; b1 into the Relu evacuation; v/o/mlp2 biases ride as
    rank-1 PSUM-preload matmuls.
  - All 128x128 block transposes (LN outputs, LM-head h) run on the
    DMA engines (InstDmaTransposeAnt, 14ns/xbar-tile) instead of the
    PE array; the normalized activations are written token-major so
    one DMA transposes 2 tiles per call.
  - Softmax over the *query* axis in transposed score layout wT[k, t]:
    one Exp per (head, key-chunk); the denominator is a pass-through
    scalar_tensor_tensor on DVE (4x bf16 mode) with fused accum_out;
    1/denom folds into v rows via an ALU-divide on the Pool engine
    (gpsimd: SBUF-only work — it has no PSUM port).
  - Causal mask via bf16 (-80)-triangle PSUM-preload matmul.
  - Per-layer weights arrive as ONE packed DMA; LM-head weights
    prefetch at the start of the last layer; the LM head is fused into
    the last MLP tile loop so its GEMMs overlap the transformer tail.
"""

import sys

sys.path.insert(0, "/opt/trn_rl_repo")

import numpy as np

import concourse.bass as bass
import concourse.mybir as mybir
import concourse.tile as tile
from concourse import bacc
from concourse import bass_utils

F32 = mybir.dt.float32
BF16 = mybir.dt.bfloat16
I32 = mybir.dt.int32
AF = mybir.ActivationFunctionType
ALU = mybir.AluOpType

V, C, T, H, HS, NL, B = 50257, 384, 1024, 6, 64, 3, 2
P = 128
N = T                      # 1024 tokens per core (one batch)
NT = N // P                # 8 token chunks
NC3 = C // P               # 3 channel chunks
NCORE = 8
NQ = 4                     # vocab quarters
VPAD = 51200               # padded vocab (4 * 12800)
VSH = VPAD // NQ           # 12800 vocab columns per core
KC = T // P                # 8 key chunks
TB = T // 512              # 2 query blocks of 512
NEG = -80.0                # mask bias (exp(-80) ~ 1.8e-35)
NW = 18                    # packed weight tiles per layer (6 mats x 3 chunks)

_CACHE: dict = {}
PHASES: list = []


def _mark(nc, label):
    PHASES.append((label, int(nc.next_id())))


class _EvacSplit:
    """Round-robin PSUM->SBUF evacuation copies over DVE / ACT.
    (gpsimd has no PSUM port, so Pool is not in this rotation.)"""

    def __init__(self, nc):
        self.nc = nc
        self.i = 0

    def copy(self, out, in_):
        self.i += 1
        if self.i % 2 == 0:
            self.nc.vector.tensor_copy(out, in_)
        else:
            self.nc.scalar.copy(out, in_)


def _build(has_blm: bool):
    nc = bacc.Bacc("TRN2", target_bir_lowering=False, debug=False)

    d_idx = nc.dram_tensor("idx", [N, 1], I32, kind="ExternalInput").ap()
    d_tok = nc.dram_tensor("tok_emb", [V, C], BF16, kind="ExternalInput").ap()
    d_pos = nc.dram_tensor("pos", [N, C], BF16, kind="ExternalInput").ap()
    # packed per-layer weights: rows (l*NW + k)*P + p, k = mat*3 + chunk
    d_wall = nc.dram_tensor("wall", [NL * NW * P, C], BF16,
                            kind="ExternalInput").ap()
    # packed per-partition bias columns: [bq | bk | b1] per layer
    d_bcol = nc.dram_tensor("bcol", [NL * C, 3], F32,
                            kind="ExternalInput").ap()
    # packed bias rows [bv ; bo ; b2] per layer
    d_brow = nc.dram_tensor("brow", [NL * 3, C], BF16,
                            kind="ExternalInput").ap()
    d_ones = nc.dram_tensor("ones", [1, 512], BF16, kind="ExternalInput").ap()
    d_identb = nc.dram_tensor("identb", [P, P], BF16,
                              kind="ExternalInput").ap()
    d_trib = nc.dram_tensor("trib", [P, P], BF16, kind="ExternalInput").ap()
    d_wlm = nc.dram_tensor("wlm", [C, VSH], BF16, kind="ExternalInput").ap()
    if has_blm:
        d_blm = nc.dram_tensor("blm", [1, VSH], BF16,
                               kind="ExternalInput").ap()
    d_out = nc.dram_tensor("logits", [N, VSH], BF16,
                           kind="ExternalOutput").ap()

    with tile.TileContext(nc) as tc:
        _emit(nc, tc, locals(), has_blm)
    nc.compile()
    return nc


def _emit(nc, tc, d, has_blm):
    from contextlib import ExitStack

    with ExitStack() as ctx:
        hpool = ctx.enter_context(tc.tile_pool(name="hpool", bufs=NT))
        pers = ctx.enter_context(tc.tile_pool(name="pers", bufs=1))
        spool = ctx.enter_context(tc.tile_pool(name="spool", bufs=8))

        # ------------- embedding gather (DMAs issued first) -------------
        _mark(nc, "embed")
        h = []  # 8 residual-stream tiles (128, 384) fp32
        idx_all = pers.tile([P, NT], I32, name="idx", tag="idx")
        nc.sync.dma_start(
            idx_all[:],
            bass.AP(tensor=d["d_idx"].tensor, offset=d["d_idx"].offset,
                    ap=[[1, P], [P, NT]]))
        posw = pers.tile([P, NT * C], BF16, name="pos", tag="pos")
        nc.sync.dma_start(
            posw[:],
            bass.AP(tensor=d["d_pos"].tensor, offset=d["d_pos"].offset,
                    ap=[[C, P], [P * C, NT], [1, C]]))

        # ------------- constants -------------
        ones = pers.tile([1, 512], BF16, name="ones", tag="ones")
        identb = pers.tile([P, P], BF16, name="identb", tag="identb")
        trib = pers.tile([P, P], BF16, name="trib", tag="trib")
        eps = pers.tile([P, 1], F32, name="eps", tag="eps")
        nc.sync.dma_start(ones[:], d["d_ones"][:])
        nc.sync.dma_start(identb[:], d["d_identb"][:])
        nc.sync.dma_start(trib[:], d["d_trib"][:])
        nc.vector.memset(eps[:], 1e-5)

        with tc.tile_pool(name="epool", bufs=3) as epool:
            for i in range(NT):
                emb = epool.tile([P, C], BF16, name="emb", tag="emb")
                nc.gpsimd.indirect_dma_start(
                    out=emb[:], out_offset=None, in_=d["d_tok"][:],
                    in_offset=bass.IndirectOffsetOnAxis(
                        ap=idx_all[:, i:i + 1], axis=0),
                )
                h_i = hpool.tile([P, C], F32, name="h", tag="h")
                nc.vector.tensor_add(h_i[:], emb[:],
                                     posw[:, i * C:(i + 1) * C])
                h.append(h_i)

        ev = _EvacSplit(nc)

        # LM-head weights: persistent tiles, prefetched in small chunks
        # from layer 1 onward so the transfers never head-of-line block
        # the LN transpose DMAs on the (serialized) DMA engines.
        lmpool = ctx.enter_context(tc.tile_pool(name="lmpool", bufs=1))
        wlm = [lmpool.tile([P, VSH], BF16, name=f"wlm{c}", tag=f"wlm{c}")
               for c in range(NC3)]
        blm = None
        if has_blm:
            blm = lmpool.tile([1, VSH], BF16, name="blm", tag="blm")

        # ------------- layers -------------
        for l in range(NL):
            with ExitStack() as lctx:
                wpool = lctx.enter_context(
                    tc.tile_pool(name=f"wpool{l}", bufs=1))
                wall = wpool.tile([P, NW * C], BF16, name="wall", tag="wall")
                r0 = l * NW * P
                for half in range(2):
                    nc.sync.dma_start(
                        wall[:, half * 9 * C:(half + 1) * 9 * C],
                        bass.AP(tensor=d["d_wall"].tensor,
                                offset=(d["d_wall"].offset
                                        + (r0 + half * 9 * P) * C),
                                ap=[[C, P], [P * C, 9], [1, C]]))

                def wslice(mat, c):
                    k = mat * 3 + c
                    return wall[:, k * C:(k + 1) * C]

                wq = [wslice(0, c) for c in range(NC3)]
                wk = [wslice(1, c) for c in range(NC3)]
                wv = [wslice(2, c) for c in range(NC3)]
                wo = [wslice(3, c) for c in range(NC3)]
                w1 = [wslice(4, c) for c in range(NC3)]
                w2 = [wslice(5, c) for c in range(NC3)]

                bcol = wpool.tile([P, NC3 * 3], F32, name="bcol", tag="bcol")
                nc.sync.dma_start(
                    bcol[:],
                    bass.AP(tensor=d["d_bcol"].tensor,
                            offset=d["d_bcol"].offset + l * C * 3,
                            ap=[[3, P], [P * 3, NC3], [1, 3]]))
                bqkt = [bcol[:, 3 * c:3 * c + 2] for c in range(NC3)]
                b1t = [bcol[:, 3 * c + 2:3 * c + 3] for c in range(NC3)]
                brow = wpool.tile([1, 3 * C], BF16, name="brow", tag="brow")
                nc.sync.dma_start(
                    brow[:],
                    bass.AP(tensor=d["d_brow"].tensor,
                            offset=d["d_brow"].offset + l * 3 * C,
                            ap=[[3 * C, 1], [1, 3 * C]]))
                bv = brow[:, 0:C]
                bo = brow[:, C:2 * C]
                b2 = brow[:, 2 * C:3 * C]

                if l == 1:
                    # prefetch LM-head weights in 12 small chunks
                    for c in range(NC3):
                        for q4 in range(4):
                            nc.sync.dma_start(
                                wlm[c][:, q4 * 3200:(q4 + 1) * 3200],
                                d["d_wlm"][c * P:(c + 1) * P,
                                           q4 * 3200:(q4 + 1) * 3200])
                    if has_blm:
                        nc.sync.dma_start(blm[:], d["d_blm"][:])

                with ExitStack() as actx:
                    attpool = actx.enter_context(
                        tc.tile_pool(name=f"attpool{l}", bufs=1))
                    attT = [attpool.tile([P, N], BF16, name=f"attT{c}",
                                         tag=f"attT{c}")
                            for c in range(NC3)]
                    with ExitStack() as qctx:
                        atp = qctx.enter_context(
                            tc.tile_pool(name=f"atp{l}", bufs=1))
                        _mark(nc, f"L{l}.ln1")
                        at = _layernorm_transposed(
                            nc, tc, h, eps, atp, spool, f"a{l}")
                        _mark(nc, f"L{l}.v")

                        vpool = qctx.enter_context(
                            tc.tile_pool(name=f"vpool{l}", bufs=NT))
                        psc = qctx.enter_context(tc.tile_pool(
                            name=f"psc{l}", bufs=2, space="PSUM"))
                        psa = qctx.enter_context(tc.tile_pool(
                            name=f"psa{l}", bufs=2, space="PSUM"))
                        v = [None] * NT

                        def build_v(i):
                            ps = psc.tile([P, C], F32, name="psc", tag="psc")
                            nc.tensor.matmul(ps[:], ones[:, :P], bv,
                                             start=True, stop=False)
                            for c in range(NC3):
                                nc.tensor.matmul(
                                    ps[:], at(c, i), wv[c], start=False,
                                    stop=(c == NC3 - 1))
                            v_i = vpool.tile([P, C], BF16, name="v", tag="v")
                            ev.copy(v_i[:], ps[:])
                            v[i] = v_i

                        qkpool = qctx.enter_context(
                            tc.tile_pool(name=f"qkpool{l}", bufs=2))
                        ppool = qctx.enter_context(
                            tc.tile_pool(name=f"ppool{l}", bufs=6))
                        vspool = qctx.enter_context(
                            tc.tile_pool(name=f"vspool{l}", bufs=8))
                        _mark(nc, f"L{l}.attn")

                        def build_qk(m):
                            qT_m = qkpool.tile([P, N], BF16, name="qT",
                                               tag="qT")
                            kT_m = qkpool.tile([P, N], BF16, name="kT",
                                               tag="kT")
                            for dst, wmat, bc in (
                                    (qT_m, wq, bqkt[m][:, 0:1]),
                                    (kT_m, wk, bqkt[m][:, 1:2])):
                                for t4 in range(N // 512):
                                    ps = psc.tile([P, 512], F32, name="psc",
                                                  tag="psc")
                                    for c in range(NC3):
                                        nc.tensor.matmul(
                                            ps[:],
                                            wmat[c][:, m * P:(m + 1) * P],
                                            at(c, t4, blk=True),
                                            start=(c == 0),
                                            stop=(c == NC3 - 1))
                                    nc.vector.scalar_tensor_tensor(
                                        dst[:, t4 * 512:(t4 + 1) * 512],
                                        ps[:], bc,
                                        dst[:, t4 * 512:(t4 + 1) * 512],
                                        op0=ALU.add, op1=ALU.bypass)
                            return qT_m, kT_m

                        for i in range(NT):
                            build_v(i)
                        qk_next = build_qk(0)
                        for m in range(NC3):
                            qT_m, kT_m = qk_next
                            qk_next = None
                            _attention_m(
                                nc, l, m, qT_m, kT_m, v, attT, trib,
                                identb, ppool, vspool, spool, psc, psa, ev,
                                mid=(lambda mm=m: build_qk(mm + 1))
                                if m + 1 < NC3 else None)
                            if m + 1 < NC3:
                                qk_next = _attention_m.qk_built

                    _mark(nc, f"L{l}.proj")
                    with tc.tile_pool(name=f"pso{l}", bufs=2,
                                      space="PSUM") as pso:
                        for i in range(NT):
                            ps = pso.tile([P, C], F32, name="pmm", tag="pmm")
                            nc.tensor.matmul(ps[:], ones[:, :P], bo,
                                             start=True, stop=False)
                            for c in range(NC3):
                                nc.tensor.matmul(
                                    ps[:], attT[c][:, i * P:(i + 1) * P],
                                    wo[c], start=False, stop=(c == NC3 - 1))
                            nc.vector.tensor_add(h[i][:], h[i][:], ps[:])

                # --- LN2 + MLP (+ fused LM head on the last layer) ---
                _mark(nc, f"L{l}.mlp")
                with ExitStack() as mctx:
                    atp2 = mctx.enter_context(
                        tc.tile_pool(name=f"atp2{l}", bufs=1))
                    m1pool = mctx.enter_context(
                        tc.tile_pool(name=f"m1pool{l}", bufs=3))
                    a2t = _layernorm_transposed(
                        nc, tc, h, eps, atp2, spool, f"b{l}")
                    if l < NL - 1:
                        psm = mctx.enter_context(tc.tile_pool(
                            name=f"psm{l}", bufs=4, space="PSUM"))
                        ps_m1 = psm
                    else:
                        pstm = mctx.enter_context(tc.tile_pool(
                            name="pstm", bufs=2, space="PSUM"))
                        pslm = mctx.enter_context(tc.tile_pool(
                            name="pslm", bufs=6, space="PSUM"))
                        ps_m1 = pslm
                    m1T = [m1pool.tile([P, N], BF16, name="m1T", tag="m1T")
                           for _ in range(NC3)]
                    for cm in range(NC3):
                        for t4 in range(N // 512):
                            ps = ps_m1.tile([P, 512], F32, name="plm",
                                            tag="plm")
                            for c in range(NC3):
                                nc.tensor.matmul(
                                    ps[:], w1[c][:, cm * P:(cm + 1) * P],
                                    a2t(c, t4, blk=True),
                                    start=(c == 0), stop=(c == NC3 - 1))
                            nc.scalar.activation(
                                m1T[cm][:, t4 * 512:(t4 + 1) * 512],
                                ps[:], AF.Relu, bias=b1t[cm][:, 0:1])

                    if l < NL - 1:
                        for i in range(NT):
                            ps = psm.tile([P, C], F32, name="pmm", tag="pmm")
                            nc.tensor.matmul(ps[:], ones[:, :P], b2,
                                             start=True, stop=False)
                            for c in range(NC3):
                                nc.tensor.matmul(
                                    ps[:], m1T[c][:, i * P:(i + 1) * P],
                                    w2[c], start=False, stop=(c == NC3 - 1))
                            nc.vector.tensor_add(h[i][:], h[i][:], ps[:])
                    else:
                        _mark(nc, "lmhead")
                        hbpool = mctx.enter_context(
                            tc.tile_pool(name="hbpool", bufs=2))
                        opool = mctx.enter_context(
                            tc.tile_pool(name="opool", bufs=2))
                        for i in range(NT):
                            ps = pstm.tile([P, C], F32, name="pmm2",
                                           tag="pmm2")
                            nc.tensor.matmul(ps[:], ones[:, :P], b2,
                                             start=True, stop=False)
                            for c in range(NC3):
                                nc.tensor.matmul(
                                    ps[:], m1T[c][:, i * P:(i + 1) * P],
                                    w2[c], start=False, stop=(c == NC3 - 1))
                            hb = hbpool.tile([P, C], BF16, name="hb",
                                             tag="hb")
                            nc.vector.tensor_add(hb[:], h[i][:], ps[:])
                            # transpose hb -> hT (3 chunks) on the DMA xbar
                            hT = hbpool.tile([P, C], BF16, name="hT",
                                             tag="hT")
                            hT_ap = bass.AP(tensor=hT.tensor,
                                            offset=hT.offset,
                                            ap=[hT.ap[0], [P, NC3], [1, P]])
                            nc.sync.dma_start_transpose(hT_ap, hb[:])
                            # 25 x 512-wide logits tiles for this token
                            # chunk, stationary operand outermost in groups
                            # of 3 (1 Ldweights per (group, c))
                            ost = opool.tile([P, VSH], BF16, name="ost",
                                             tag="ost")
                            nvg = VSH // 512
                            for g0 in range(0, nvg, 3):
                                vgs = range(g0, min(g0 + 3, nvg))
                                pvs = {}
                                for vg in vgs:
                                    pvs[vg] = pslm.tile([P, 512], F32,
                                                        name="plm",
                                                        tag="plm")
                                    if has_blm:
                                        nc.tensor.matmul(
                                            pvs[vg][:], ones[:, :P],
                                            blm[:, vg * 512:(vg + 1) * 512],
                                            start=True, stop=False)
                                for c in range(NC3):
                                    for vg in vgs:
                                        nc.tensor.matmul(
                                            pvs[vg][:],
                                            hT[:, c * P:(c + 1) * P],
                                            wlm[c][:,
                                                   vg * 512:(vg + 1) * 512],
                                            start=(c == 0 and not has_blm),
                                            stop=(c == NC3 - 1))
                                for vg in vgs:
                                    ev.copy(ost[:, vg * 512:(vg + 1) * 512],
                                            pvs[vg][:])
                            nc.sync.dma_start(
                                d["d_out"][i * P:(i + 1) * P, :], ost[:])


def _layernorm_transposed(nc, tc, h, eps, atpool, spool, label):
    """LN (affine folded into weights host-side); the normalized
    activations are written token-major into aW and transposed by the
    DMA xbar into aT (24 x 128x128 blocks at stride 128). Returns an
    accessor at(c, i) -> [128,128] chunk-c block of token tile i, or
    at(c, t4, blk=True) -> [128, 512] strided 4-block moving operand."""
    mv = spool.tile([P, 2 * NT], F32, name="mv", tag="mv")
    for i in range(NT):
        st = spool.tile([P, 6], F32, name="st", tag="st")
        nc.vector.bn_stats(st[:], h[i][:])
        nc.vector.bn_aggr(mv[:, 2 * i:2 * i + 2], st[:])
    std = spool.tile([P, NT], F32, name="std", tag="std")
    rstd = spool.tile([P, NT], F32, name="rstd", tag="rstd")
    nmr = spool.tile([P, NT], F32, name="nmr", tag="nmr")
    for g in range(0, NT, 4):
        nc.scalar.activation(std[:, g:g + 4], mv[:, 2 * g + 1:2 * g + 8:2],
                             AF.Sqrt, bias=eps[:, :1])
        nc.vector.reciprocal(rstd[:, g:g + 4], std[:, g:g + 4])
        nc.vector.scalar_tensor_tensor(nmr[:, g:g + 4],
                                       mv[:, 2 * g:2 * g + 8:2], -1.0,
                                       rstd[:, g:g + 4],
                                       op0=ALU.mult, op1=ALU.mult)
    aW = atpool.tile([P, NT * C], BF16, name=f"aW{label}", tag="aW")
    aTw = atpool.tile([P, NT * C], BF16, name=f"aT{label}", tag="aT")
    for i in range(NT):
        nc.vector.tensor_scalar(aW[:, i * C:(i + 1) * C], h[i][:],
                                rstd[:, i:i + 1], nmr[:, i:i + 1],
                                op0=ALU.mult, op1=ALU.add)
    for i2 in range(NT // 2):
        out_ap = bass.AP(tensor=aTw.tensor,
                         offset=aTw.offset + i2 * 2 * C,
                         ap=[aTw.ap[0], [P, 2 * NC3], [1, P]])
        nc.scalar.dma_start_transpose(out_ap, aW[:, i2 * 2 * C:
                                                 (i2 + 1) * 2 * C])

    def at(c, i, blk=False):
        if blk:  # 512 tokens: 4 tiles of 128 at stride C
            return bass.AP(tensor=aTw.tensor,
                           offset=aTw.offset + i * 4 * C + c * P,
                           ap=[aTw.ap[0], [C, 4], [1, P]])
        return bass.AP(tensor=aTw.tensor,
                       offset=aTw.offset + i * C + c * P,
                       ap=[aTw.ap[0], [1, P]])

    return at


def _attention_m(nc, l, m, qT_m, kT_m, v, attT, trib, identb,
                 ppool, vspool, spool, psc, psa, ev, mid=None):
    """Scores + query-axis softmax + p@v for heads (2m, 2m+1).

    Scores for one (head, key-chunk) land in a (128, 1024) two-bank PSUM
    tile; one Exp covers the causally-valid range [128*kc : 1024). The
    denominator comes from a pass-through STT on DVE (4x bf16) with
    fused accum_out; v rows are divided by it on the Pool engine. p@v
    accumulates in (64, 1024) PSUM tiles, two heads in separate free
    halves, pipelined 2 key-chunks behind the scores."""
    d0 = spool.tile([P, 16], F32, name="d0", tag="d0")
    dinv = spool.tile([P, 16], F32, name="dinv", tag="dinv")
    dsc = spool.tile([P, T], BF16, name="dsc", tag="dsc")

    att_ps = {tb: psa.tile([64, 1024], F32, name="patt", tag="patt")
              for tb in range(TB)}
    pending = []

    for kc in range(KC):
        p_kc = ppool.tile([P, 2 * T], BF16, name="p", tag="p")
        lo_kc = 128 * kc
        w_kc = T - lo_kc
        diag_tb = kc // (512 // P)
        for hh in range(2):
            pp = psc.tile([P, T], F32, name="psc", tag="psc")
            nc.tensor.matmul(pp[:, lo_kc:lo_kc + P], identb[:], trib[:],
                             start=True, stop=False)
            for tb in range(TB):
                lo = 128 * kc - 512 * tb
                if lo >= 512:
                    continue
                lo = max(lo, 0)
                nc.tensor.matmul(
                    pp[:, tb * 512 + lo:(tb + 1) * 512],
                    kT_m[64 * hh:64 * hh + 64, lo_kc:lo_kc + P],
                    qT_m[64 * hh:64 * hh + 64,
                         tb * 512 + lo:(tb + 1) * 512],
                    start=(tb != diag_tb), stop=(tb == TB - 1))
            nc.scalar.activation(
                p_kc[:, hh * T + lo_kc:(hh + 1) * T],
                pp[:, lo_kc:T], AF.Exp)
            # denominator: pass-through tensor_scalar (4x bf16 on DVE)
            # with fused row-sum accum
            nc.vector.tensor_scalar(
                dsc[:, :w_kc], p_kc[:, hh * T + lo_kc:(hh + 1) * T],
                0.0, 0.0, op0=ALU.add, op1=ALU.add,
                accum_out=d0[:, 8 * hh + kc:8 * hh + kc + 1])

        vs = vspool.tile([P, P], BF16, name="vs", tag="vs")
        for hh in range(2):
            vslice = v[kc][:, m * P + 64 * hh:m * P + 64 * hh + 64]
            nc.vector.scalar_tensor_tensor(
                vs[:, 64 * hh:64 * hh + 64], vslice,
                d0[:, 8 * hh + kc:8 * hh + kc + 1], vslice,
                op0=ALU.divide, op1=ALU.bypass)
        pending.append((kc, p_kc, vs))
        if len(pending) > 3:
            _emit_att(nc, attT, att_ps, m, *pending.pop(0))

    if mid is not None:
        _attention_m.qk_built = mid()
    while pending:
        _emit_att(nc, attT, att_ps, m, *pending.pop(0))
    nc.vector.tensor_copy(attT[m][0:64, 512:1024], att_ps[1][:, 0:512])
    nc.vector.tensor_copy(attT[m][64:128, 512:1024],
                          att_ps[1][:, 512:1024])


def _emit_att(nc, attT, att_ps, m, kc, p_kc, vs):
    for tb in range(TB):
        lo = 128 * kc - 512 * tb
        if lo >= 512:
            continue
        lo = max(lo, 0)
        last = (kc == (3 if tb == 0 else KC - 1))
        for hh in range(2):
            nc.tensor.matmul(
                att_ps[tb][:, hh * 512 + lo:(hh + 1) * 512],
                vs[:, 64 * hh:64 * hh + 64],
                p_kc[:, hh * T + tb * 512 + lo:hh * T + (tb + 1) * 512],
                start=(kc == 0), stop=last, skip_group_check=True)
    if kc == 3:
        nc.vector.tensor_copy(attT[m][0:64, 0:512], att_ps[0][:, 0:512])
        nc.vector.tensor_copy(attT[m][64:128, 0:512],
                              att_ps[0][:, 512:1024])


# ---------------------------------------------------------------------------
# host side
# ---------------------------------------------------------------------------

def _prep_inputs(inputs):
    import ml_dtypes
    f32 = np.float32
    bf16 = ml_dtypes.bfloat16
    tok_emb = np.asarray(inputs["tok_emb"], f32)
    pos_emb = np.asarray(inputs["pos_emb"], f32)
    x = np.asarray(inputs["x"]).astype(np.int32)  # (B, T)

    def fold_qkv(W, bias, g, b_ln, extra=1.0):
        Wf = np.transpose(np.asarray(W, f32), (0, 2, 1, 3)).reshape(NL, C, C)
        bf = (np.asarray(bias, f32).reshape(NL, C)
              + np.einsum("lc,lcd->ld", np.asarray(b_ln, f32), Wf))
        Wg = Wf * np.asarray(g, f32)[:, :, None]
        return (Wg * extra), (bf * extra)

    g1, b1n = inputs["ln1_g"], inputs["ln1_b"]
    g2, b2n = inputs["ln2_g"], inputs["ln2_b"]
    wq, bq = fold_qkv(inputs["Wq"], inputs["bq"], g1, b1n)
    wk, bk = fold_qkv(inputs["Wk"], inputs["bk"], g1, b1n, extra=HS ** -0.5)
    wv, bv = fold_qkv(inputs["Wv"], inputs["bv"], g1, b1n)

    W1 = np.asarray(inputs["W1"], f32)
    w1 = W1 * np.asarray(g2, f32)[:, :, None]
    b1f = (np.asarray(inputs["b1"], f32)
           + np.einsum("lc,lcd->ld", np.asarray(b2n, f32), W1))
    wo = np.asarray(inputs["Wo"], f32).reshape(NL, C, C)
    w2 = np.asarray(inputs["W2"], f32).reshape(NL, C, C)

    wall = np.stack([wq, wk, wv, wo, w1, w2], axis=1)  # (NL, 6, C, C)
    wall = wall.reshape(NL * NW * P, C).astype(bf16)

    bcol = np.stack([bq.reshape(-1), bk.reshape(-1), b1f.reshape(-1)],
                    axis=1).astype(f32)  # (NL*C, 3)
    brow = np.stack([bv, np.asarray(inputs["bo"], f32),
                     np.asarray(inputs["b2"], f32)], axis=1)  # (NL, 3, C)
    brow = brow.reshape(NL * 3, C).astype(bf16)

    tri = np.zeros((P, P), f32)
    tri[np.tril_indices(P, -1)] = NEG  # tri[k, t] = NEG where t < k
    trib = tri.astype(bf16)
    identb = np.eye(P, dtype=bf16)

    wlm_pad = np.zeros((C, VPAD), f32)
    wlm_pad[:, :V] = np.asarray(inputs["Wlm"], f32)
    blm_pad = np.zeros((1, VPAD), f32)
    blm_pad[0, :V] = np.asarray(inputs["blm"], f32)
    has_blm = bool(np.any(blm_pad))

    common = {
        "tok_emb": tok_emb.astype(bf16),
        "pos": pos_emb.astype(bf16),
        "wall": wall,
        "bcol": bcol,
        "brow": brow,
        "ones": np.ones((1, 512), bf16),
        "identb": identb,
        "trib": trib,
    }
    in_maps = []
    for j in range(NCORE):
        b, q = divmod(j, NQ)
        im = dict(common)
        im["idx"] = np.ascontiguousarray(x[b].reshape(N, 1))
        im["wlm"] = np.ascontiguousarray(
            wlm_pad[:, q * VSH:(q + 1) * VSH]).astype(bf16)
        if has_blm:
            im["blm"] = np.ascontiguousarray(
                blm_pad[:, q * VSH:(q + 1) * VSH]).astype(bf16)
        in_maps.append(im)
    return in_maps, has_blm


def kernel(**inputs):
    in_maps, has_blm = _prep_inputs(inputs)
    key = ("nc", has_blm)
    if key not in _CACHE:
        _CACHE[key] = _build(has_blm)
    nc = _CACHE[key]
    res = bass_utils.run_bass_kernel_spmd(nc, in_maps,
                                          core_ids=list(range(NCORE)))
    logits = np.zeros((B, T, VPAD), np.float32)
    for j in range(NCORE):
        b, q = divmod(j, NQ)
        logits[b, :, q * VSH:(q + 1) * VSH] = \
            np.asarray(res.results[j]["logits"], np.float32)
    return logits[:, :, :V]


if __name__ == "__main__":
    pass
